# revision 25
# baseline (speedup 1.0000x reference)
"""Cross-attention Trainium2 Bass kernel (v2).

Reference computation (per batch b):
    q = relu(scale_q * (Wq @ qf) + bias_q)          [C, Nq]
    k = relu(scale_k * (Wk @ kf) + bias_k)          [C, Nk]
    v = relu(scale_v * (Wv @ kf) + bias_v)          [C, Nk]
    sim  = q.T @ k / sqrt(C)                        [Nq, Nk]
    attn = softmax(sim, axis=-1)
    ctx  = v @ attn.T                               [C, Nq]

Sharding: 8 cores = 4 batches x 2 query halves (Nq 4096 -> 2048 per core).

Calibrated HW model (microbench, TRN2):
  - PE streams 1 output column/cycle at 2.4GHz regardless of dtype;
    fp8 DoubleRow contracts 256 rows/stream. 512-col matmul = 216ns,
    per-instruction floor ~127ns. Clock ramps 0.65->1.2->2.4GHz over
    ~3us of CONTINUOUS execution; idle gaps drop it back.
  - ACT: 0.83ns/elem/lane + 260ns/instr; accumulate output free.
  - DVE: 1 el/cyc/lane from PSUM, 2 from SBUF @0.96GHz.
    tensor_scalar [128,1024] from PSUM = 1223ns (~ACT's 1114).
  - reciprocal 6.5ns/free-elem; reciprocal_approx_fast 6x faster.

v2 design (changes vs v1):
  - PE warmup: a chain of tiny junk matmuls issued at t=0 (no DMA deps)
    keeps the PE busy through the input-DMA window so the clock is at
    2.4GHz when the real work starts.
  - exp split: 12 of 16 sim pairs per chunk exp'd on ACT; 4 on DVE via
    a Schraudolph fp8 trick: u8 = round(sim*A8 + B8) bit-cast as
    float8e4 IS exp(sim*scale+shift) to ~4% (vs fp8's inherent ~2%).
  - v^T bias: a K=1 ones-matmul pre-loads bias_v into the psv PSUM
    (start=True), the two kf matmuls accumulate on top, and a single
    relu drains it -- the per-pair DVE broadcast-ADD pass is gone.
  - reciprocal_approx_fast for the softmax denominators.
  - sums matmuls: unchanged ones-channel trick (PE column-rate bound
    at 16x216ns/chunk; no cheaper engine exists for partition sums).
"""

import sys

for _p in ("/opt/trn_rl_repo", "/root/.axon_site/_ro/trn_rl_repo"):
    if _p not in sys.path:
        sys.path.insert(0, _p)

import numpy as np

import concourse.bacc as bacc
import concourse.mybir as mybir
import concourse.tile as tile
from concourse.bass_utils import run_bass_kernel_spmd

F32 = mybir.dt.float32
F32R = mybir.dt.float32r
F8 = mybir.dt.float8e4
U8 = mybir.dt.uint8
AF = mybir.ActivationFunctionType
DR = mybir.MatmulPerfMode.DoubleRow

B, C, H, W = 4, 256, 64, 64
NK = H * W          # 4096 key positions per batch
NQ = NK // 2        # 2048 query positions per core
P = 128
CO = C // P         # 2 contraction subtiles
QC = 512            # query chunk (matmul moving free dim)
NQC = NQ // QC      # 4 query chunks per core
KT = NK // P        # 32 key tiles
NP = KT // 2        # 16 key-tile pairs
VP = 272            # vT row pitch (C+2 used, padded for DoubleRow stride)
LAG = 7             # sums matmul lag (in pairs) behind its exp tile
EXP_SHIFT = -4.0    # exp(sim/sqrt(C) + EXP_SHIFT); sim/sqrt(C) in [0.5, 7.5]
SCALE = 1.0 / np.sqrt(C)
LOG2E = 1.4426950408889634
A8 = float(SCALE * LOG2E * 8.0)
# +56 biases into the f8e4m3 exponent; -0.44 recenters the Schraudolph
# overestimate (mean +3.9% measured) around zero.
B8 = float(56.0 + EXP_SHIFT * LOG2E * 8.0 - 0.44)
DVE_PAIRS = (2, 6, 10, 14)   # sim pairs exp'd on DVE instead of ACT (steady)
DVE_PAIRS0 = (1, 3, 5, 7, 9, 11, 13, 15)  # chunk 0 (ACT is relu-loaded)


def _build_program():
    nc = bacc.Bacc("TRN2", target_bir_lowering=False, debug=False)

    # All inputs are pre-packed host-side into the SBUF layouts so every
    # DMA is a contiguous per-partition block (fewest descriptors), and
    # the weight/bias blobs ride the Activation HWDGE queue while the
    # big qf/kf streams ride SP -- the two queues dispatch in parallel.
    qf = nc.dram_tensor("qfp", [P, CO * NQ], F8, kind="ExternalInput").ap() \
        .rearrange("p (co n) -> p co n", co=CO)
    kf = nc.dram_tensor("kfp", [P, CO * NK], F8, kind="ExternalInput").ap() \
        .rearrange("p (co n) -> p co n", co=CO)
    wqp = nc.dram_tensor("wqp", [P, CO * C], F8, kind="ExternalInput").ap() \
        .rearrange("p (co o) -> p co o", co=CO)
    wkv = nc.dram_tensor("wkv", [P, CO * (C + VP)], F8,
                         kind="ExternalInput").ap() \
        .rearrange("p (co o) -> p co o", co=CO)
    bqk = nc.dram_tensor("bqk", [P, 2 * CO], F32, kind="ExternalInput").ap() \
        .rearrange("p (a co) -> p a co", a=2)
    bvb = nc.dram_tensor("bvb", [1, C + 2], F8, kind="ExternalInput").ap()
    out = nc.dram_tensor("out", [C, NQ], F32, kind="ExternalOutput").ap()
    out_t = out.rearrange("(co ci) n -> ci co n", ci=P)

    with tile.TileContext(nc) as tc:
        with (
            nc.allow_low_precision(reason="fp8/fp32r matmul operands"),
            tc.tile_pool(name="consts", bufs=1) as consts,
            tc.tile_pool(name="persist", bufs=1) as persist,
        ):
            # ---- PE warmup: FAT junk matmuls with no DMA deps. The PE
            # DVFS ramp tracks array utilization, not queue busyness --
            # tiny matmuls leave it at 1.2GHz, full 512-col streams get
            # it to 2.4GHz in ~4.5us, right as the input DMAs land.
            # wj MUST be zeroed: warming up on uninitialized (high
            # entropy) operands raises PE switching power and the power
            # manager then caps the clock ~20% below peak for the WHOLE
            # kernel. All-zero operands ramp the DVFS at minimal power.
            # The memset is split across DVE and GpSimd to halve its
            # latency before the first warmup matmul can issue.
            wj = consts.tile([P, 2, QC], F8, name="wj")
            nc.vector.memset(wj[:], 0.0)
            with tc.tile_pool(name="warm_ps", bufs=1, space="PSUM") as wps:
                for i in range(10):
                    t = wps.tile([P, QC], F32, tag="w", bufs=4, name=f"wm{i}")
                    nc.tensor.matmul(t[:], wj[:, :, :P], wj[:], start=True,
                                     stop=True, perf_mode=DR)

            # ---- constants: weight/bias blobs on the ACT HWDGE queue,
            # dispatched before ACT's table-load/warm activation ----
            wqT_sb = consts.tile([P, CO, C], F8, name="wqT_sb")
            wkv_sb = consts.tile([P, CO, C + VP], F8, name="wkv_sb")
            bqk_sb = consts.tile([P, 2, CO], F32, name="bqk_sb")
            bvb_sb = consts.tile([1, C + 2], F8, name="bvb_sb")
            nc.scalar.dma_start(bqk_sb[:], bqk[:])
            nc.scalar.dma_start(wqT_sb[:], wqp[:])
            nc.scalar.dma_start(wkv_sb[:], wkv[:])
            nc.scalar.dma_start(bvb_sb[:], bvb[:])
            wkT_sb = wkv_sb[:, :, :C]
            wvT_sb = wkv_sb[:, :, C:]
            bq_sb = bqk_sb[:, 0, :]
            bk_sb = bqk_sb[:, 1, :]
            ones8_sb = consts.tile([1, P], F8, name="ones8_sb")
            nc.vector.memset(ones8_sb[:], 1.0)
            b0_sb = consts.tile([P, 1], F32, name="b0_sb")
            nc.vector.memset(b0_sb[:], EXP_SHIFT)
            # dummy activation: pulls the ~1.3us LoadActFuncSet into the
            # initial DMA-wait window instead of blocking the first relu
            warm_sb = consts.tile([P, 1], F32, name="warm_sb")
            nc.scalar.activation(warm_sb[:], b0_sb[:], AF.Relu)

            # ---- persistent activations (fp8 for DoubleRow attention) ----
            q_sb = persist.tile([P, CO, NQ], F8, name="q_sb")
            k_sb = persist.tile([P, CO, NK], F8, name="k_sb")
            vT_sb = persist.tile([P, KT, VP], F8, name="vT_sb")

            # ---- fused projections + chunk-0 attention ----
            with (
                tc.tile_pool(name="expp", bufs=1) as expp,
                tc.tile_pool(name="outp", bufs=1) as outp,
                tc.tile_pool(name="sim_ps", bufs=1, space="PSUM") as sim_ps,
            ):
                attn_ps = None
                et = {}                 # (s, kp) -> fp8 exp pair tile
                sums_ps_l = {}          # chunk -> [2, QC] sums PSUM tile
                bc_sb_l = {}

                def emit_sim_pair(s, kp):
                    qs = slice(s * QC, (s + 1) * QC)
                    ps = sim_ps.tile([P, 2, QC], F32, tag="sim", bufs=2,
                                      name=f"pss_{s}_{kp}")
                    for half in range(2):
                        kt = 2 * kp + half
                        nc.tensor.matmul(
                            ps[:, half, :],
                            k_sb[:, :, kt * P:(kt + 1) * P],
                            q_sb[:, :, qs],
                            start=True, stop=True, perf_mode=DR,
                        )
                    e = expp.tile([P, 2, QC], F8, tag="expT", bufs=20,
                                  name=f"expT_{s}_{kp}")
                    if kp in (DVE_PAIRS0 if s == 0 else DVE_PAIRS):
                        # Schraudolph fp8 exp on DVE: the fp32 affine
                        # result, saturate-rounded to u8, bit-cast as
                        # f8e4m3, IS exp(sim*SCALE+EXP_SHIFT) to ~4%.
                        nc.vector.tensor_scalar(
                            e[:].bitcast(U8), ps[:], A8, B8,
                            mybir.AluOpType.mult, mybir.AluOpType.add)
                    else:
                        nc.scalar.activation(e[:], ps[:], AF.Exp,
                                             bias=b0_sb[:], scale=float(SCALE))
                    et[(s, kp)] = e

                def emit_sums(s, j):
                    if j == 0:
                        sums_ps_l[s] = attn_ps.tile([2, QC], F32, tag="sums",
                                                    bufs=1, name=f"sums_{s}")
                    nc.tensor.matmul(
                        sums_ps_l[s][:],
                        vT_sb[:, 2 * j:2 * j + 2, C:C + 2],
                        et[(s, j)][:],
                        start=(j == 0), stop=(j == NP - 1),
                        perf_mode=DR, skip_group_check=True,
                    )

                def emit_ctx(s, kp, ctx_ps):
                    for ct in range(CO):
                        nc.tensor.matmul(
                            ctx_ps[ct][:],
                            vT_sb[:, 2 * kp:2 * kp + 2, ct * P:(ct + 1) * P],
                            et[(s, kp)][:],
                            start=(kp == 0), stop=(kp == NP - 1),
                            perf_mode=DR, skip_group_check=True,
                        )

                def emit_rcp_copy(s):
                    # raw sums row PSUM -> SBUF on DVE (ACT is busy w/ exp)
                    rcp = outp.tile([1, QC], F32, tag="rcp", bufs=2,
                                    name=f"rcp_{s}")
                    nc.vector.tensor_copy(out=rcp[:],
                                          in_=sums_ps_l.pop(s)[0:1, :])
                    return rcp

                def emit_bc(s, rcp):
                    # broadcast the raw sums row to 128 partitions on the
                    # (idle) GpSimd engine, then one fast approx
                    # reciprocal on DVE (546ns vs 3353ns exact).
                    bc_sb = outp.tile([P, QC], F32, tag="bcs", bufs=2,
                                      name=f"bc_{s}")
                    bcr = outp.tile([P, QC], F32, tag="bcr", bufs=2,
                                    name=f"bcr_{s}")
                    nc.gpsimd.partition_broadcast(bcr[:], rcp[:])
                    nc.vector.reciprocal_approx_fast(out=bc_sb[:], in_=bcr[:])
                    bc_sb_l[s] = bc_sb

                def emit_out(s, ctx_ps):
                    qs = slice(s * QC, (s + 1) * QC)
                    for ct in range(CO):
                        ot = outp.tile([P, QC], F32, tag="out", bufs=3,
                                       name=f"out_{s}_{ct}")
                        nc.vector.tensor_mul(ot[:], ctx_ps[ct][:],
                                             bc_sb_l.pop(s)[:] if ct == CO - 1
                                             else bc_sb_l[s][:])
                        # ct0 on SP, ct1 on the ACT HWDGE queue: the two
                        # dispatches (and the final completion waits)
                        # overlap instead of serializing on SP.
                        eng = nc.sync if ct == 0 else nc.scalar
                        eng.dma_start(out_t[:, ct, qs], ot[:])

                with (
                    tc.tile_pool(name="staging", bufs=1) as staging,
                    tc.tile_pool(name="proj_ps", bufs=1, space="PSUM") as proj_ps,
                ):
                    # Input DMA plan: few DMAs (~650ns serial dispatch
                    # each), need-before order.
                    qf_sb = staging.tile([P, CO, NQ], F8, name="qf_sb")
                    kf_sb = staging.tile([P, CO, NK], F8, name="kf_sb")
                    nc.sync.dma_start(qf_sb[:, :, :QC], qf[:, :, :QC])
                    nc.sync.dma_start(kf_sb[:, :, :QC], kf[:, :, :QC])
                    nc.sync.dma_start(kf_sb[:, :, QC:4 * QC], kf[:, :, QC:4 * QC])
                    nc.sync.dma_start(kf_sb[:, :, 4 * QC:], kf[:, :, 4 * QC:])
                    nc.sync.dma_start(qf_sb[:, :, QC:], qf[:, :, QC:])

                    def proj_iter(j, w_sb, bias_sb, dst, src_sb):
                        # one [*, QC] chunk of a q/k projection; relu+bias
                        # for oo=0 on ACT, oo=1 on DVE
                        for oo in range(CO):
                            ps = proj_ps.tile([P, QC], F32, tag="pj", bufs=2,
                                              name=f"ps_{j}_{oo}")
                            nc.tensor.matmul(
                                ps[:],
                                w_sb[:, :, oo * P:(oo + 1) * P],
                                src_sb[:, :, j * QC:(j + 1) * QC],
                                start=True, stop=True, perf_mode=DR,
                            )
                            if oo == 0:
                                nc.scalar.activation(
                                    dst[:, oo, j * QC:(j + 1) * QC], ps[:],
                                    AF.Relu, bias=bias_sb[:, oo:oo + 1],
                                )
                            else:
                                nc.vector.tensor_scalar(
                                    dst[:, oo, j * QC:(j + 1) * QC], ps[:],
                                    bias_sb[:, oo:oo + 1], 0.0,
                                    mybir.AluOpType.add, mybir.AluOpType.max,
                                )

                    def vt_kt(kt):
                        # vT = relu(kf.T @ Wv'.T + bias_v): [n, o] with n on
                        # partitions. bias_v (free-dim varying) is seeded
                        # into PSUM by a K=1 ones-matmul (the bias row is
                        # the moving operand), the kf matmul accumulates
                        # on top, and one relu drains PSUM -> vT_sb.
                        # One key tile per 1-bank psv tile; bufs=2 keeps
                        # the WAR on the drain off the PE's critical path.
                        psv = proj_ps.tile([P, QC], F32, tag="pv", bufs=2,
                                           name=f"psv_{kt}")
                        nc.tensor.matmul(
                            psv[:, :C + 2],
                            ones8_sb[:],
                            bvb_sb[:],
                            start=True, stop=False, skip_group_check=True,
                        )
                        nc.tensor.matmul(
                            psv[:, :C + 2],
                            kf_sb[:, :, kt * P:(kt + 1) * P],
                            wvT_sb[:, :, :C + 2],
                            start=False, stop=True,
                            perf_mode=DR, skip_group_check=True,
                        )
                        if kt % 2 == 0:
                            nc.scalar.activation(
                                vT_sb[:, kt, :C + 2],
                                psv[:, :C + 2], AF.Relu)
                        else:
                            nc.vector.tensor_scalar_max(
                                vT_sb[:, kt, :C + 2],
                                psv[:, :C + 2], 0.0)

                    proj_iter(0, wqT_sb, bq_sb, q_sb, qf_sb)
                    for j in range(NK // QC):
                        proj_iter(j, wkT_sb, bk_sb, k_sb, kf_sb)
                        emit_sim_pair(0, 2 * j)
                        emit_sim_pair(0, 2 * j + 1)
                        for kt in range(4 * j, 4 * j + 4):
                            vt_kt(kt)
                        if 4 <= j < 4 + NQ // QC - 1:
                            proj_iter(j - 3, wqT_sb, bq_sb, q_sb, qf_sb)

                attn_cm = tc.tile_pool(name="attn_ps", bufs=1,
                                       space="PSUM")
                attn_ps = attn_cm.__enter__()
                # ---- steady-state attention steps 1..NQC ----
                for step in range(1, NQC + 1):
                    s_sim = step if step < NQC else None
                    s_ctx = step - 1
                    trail = NP if s_ctx == 0 else LAG
                    base = NP - trail
                    tslots = (trail + 1) // 2
                    rcp = None
                    ctx_ps = None

                    def get_ctx_ps():
                        nonlocal ctx_ps
                        if ctx_ps is None:
                            ctx_ps = [
                                attn_ps.tile([P, QC], F32, tag="ctx", bufs=3,
                                             name=f"psc_{s_ctx}_{ct}")
                                for ct in range(CO)
                            ]
                        return ctx_ps
                    for kp in range(NP):
                        if s_sim is not None:
                            emit_sim_pair(s_sim, kp)
                        emit_ctx(s_ctx, kp, get_ctx_ps())
                        if s_sim is not None and kp >= LAG:
                            emit_sums(s_sim, kp - LAG)
                        if 2 <= kp < tslots + 2:
                            tj = base + 2 * (kp - 2)
                            emit_sums(s_ctx, tj)
                            if tj + 1 < NP:
                                emit_sums(s_ctx, tj + 1)
                            if tj + 2 >= NP:
                                rcp = emit_rcp_copy(s_ctx)
                        if kp == tslots + 3:
                            emit_bc(s_ctx, rcp)
                    for kp in range(NP):
                        et.pop((s_ctx, kp))
                    emit_out(s_ctx, ctx_ps)
                attn_cm.__exit__(None, None, None)

    nc.compile()
    return nc


_PROGRAM = None


def _get_program():
    global _PROGRAM
    if _PROGRAM is None:
        _PROGRAM = _build_program()
    return _PROGRAM


def _prepare_in_maps(
    query_feats, key_feats, Wq, Wk, Wv,
    scale_q, bias_q, scale_k, bias_k, scale_v, bias_v,
):
    import ml_dtypes
    f8 = ml_dtypes.float8_e4m3   # IEEE-style e4m3, max 240 == TRN fp8e4
    f32 = np.float32
    qf_all = np.asarray(query_feats, f32).reshape(B, C, NK)
    kf_all = np.asarray(key_feats, f32).reshape(B, C, NK)

    def pack(m, width):
        # [C, width] -> [P, CO*width] in the SBUF (ci, co, o) layout
        return np.ascontiguousarray(
            m.reshape(CO, P, width).transpose(1, 0, 2).reshape(P, CO * width))

    wqT = (np.asarray(scale_q, f32)[:, None] * np.asarray(Wq, f32)).T.astype(f8)
    wkT = (np.asarray(scale_k, f32)[:, None] * np.asarray(Wk, f32)).T.astype(f8)
    wvT = np.zeros((C, VP), f8)
    wvT[:, :C] = (np.asarray(scale_v, f32)[:, None] * np.asarray(Wv, f32)).T.astype(f8)
    wqp = pack(wqT, C)
    wkv = np.ascontiguousarray(np.concatenate(
        [wkT.reshape(CO, P, C).transpose(1, 0, 2),
         wvT.reshape(CO, P, VP).transpose(1, 0, 2)], axis=2,
    ).reshape(P, CO * (C + VP)))
    bq2 = np.asarray(bias_q, f32).reshape(CO, P).T
    bk2 = np.asarray(bias_k, f32).reshape(CO, P).T
    bqk = np.ascontiguousarray(np.stack([bq2, bk2], axis=1).reshape(P, 2 * CO))
    # bias_v row (+ ones channel at col C) for the K=1 bias matmuls
    bvb = np.zeros((1, C + 2), f8)
    bvb[0, :C] = np.asarray(bias_v, f32).astype(f8)
    bvb[0, C] = 1.0

    shared = dict(wqp=wqp, wkv=wkv, bqk=bqk, bvb=bvb)
    in_maps = []
    for core in range(8):
        b, h = divmod(core, 2)
        qfc = qf_all[b][:, h * NQ:(h + 1) * NQ].astype(f8)
        kfc = kf_all[b].astype(f8)
        in_maps.append(dict(
            qfp=pack(qfc, NQ),
            kfp=pack(kfc, NK),
            **shared,
        ))
    return in_maps


def run(inputs: dict, trace: bool = False):
    """Compile (cached) + run on 8 cores. Returns (output, BassKernelResults)."""
    nc = _get_program()
    in_maps = _prepare_in_maps(**inputs)
    res = run_bass_kernel_spmd(nc, in_maps, core_ids=list(range(8)), trace=trace)
    full = np.empty((B, C, NK), np.float32)
    for core in range(8):
        b, h = divmod(core, 2)
        full[b][:, h * NQ:(h + 1) * NQ] = res.results[core]["out"]
    return full.reshape(B, C, H, W), res


def kernel(**inputs) -> np.ndarray:
    return run(inputs)[0]
